# revision 26
# baseline (speedup 1.0000x reference)
"""Trainium2 Bass kernel for the channel-attention module.

Reference computation (per batch item, C=256 channels, N=4096 pixels):
    q = wq@x + bq; k = wk@x + bk; v = wv@x + bv          (1x1 convs)
    energy = q @ k^T                 [C, C]
    attn = softmax(energy, -1)
    out = attn @ v                   [C, N]
    y = gamma*out + x

Algorithm (algebraically identical, far less PE work):
    G' = [[x x^T, s], [s^T, N]]  (s = row sums of x)  -- Gram matrix, 257x257
    energy = wq' G' wk'^T   where wq' = [wq | bq], wk' = [wk | bk]
    attn = softmax(energy)
    out_dev = 16*(attn wv) x + 16*(attn bv) 1^T    (returned fp16)
    y = (gamma/16)*out_dev + x                      (host, fp32)

Key layout/precision choices:
  * Host supplies x^T pre-transposed with a ones column appended, so the
    Gram matmul also produces the row sums s (no DVE reduce, no DMA
    transpose on device).
  * All input DMAs go on ONE queue in need-order (xt0 chunks, weights,
    xt1 chunks, x8): a single queue runs near the full per-core HBM
    rate, so the first gram chunk lands early and gram paces behind the
    feed instead of waiting for everything.
  * Gram is symmetric: row-block 1 is computed only for cols 128:258 and
    the (1,0) block is reconstructed with one PE transpose. Each
    accumulation group gets its own PSUM bank (HW start-flag zeroing is
    bank-granular).
  * The final (attn wv) @ x matmul runs in fp8-e4m3 with DoubleRow perf
    mode (256-deep contraction per instruction, 2x PE rate). (attn wv)
    is scaled by 16 before the fp8 cast so its entries sit in e4m3's
    normal range; the host divides by 16 (exact). x is sent as a
    separate fp8 copy. The x passthrough and gamma scaling happen on
    the host in fp32, so fp8 never touches the dominant x term.
  * Emission order pipelines the two items: item0's T/E run before
    item1's gram on the PE queue; item0's softmax (DVE/ACT) hides under
    item1's gram; item0's output matmuls hide item1's softmax.
  * Output drain: per [128,1024] PSUM tile, the two 512-halves are
    copied (+bias, fp16 cast) by ACT and DVE in parallel, then one
    256KB store per tile.

Sharding: data-parallel over batch B=16 across 8 cores (2 items/core).
"""

import os
import sys

sys.path.insert(0, "/opt/trn_rl_repo")

from contextlib import ExitStack

import ml_dtypes
import numpy as np

import concourse.bacc as bacc
import concourse.tile as tile
from concourse import mybir
from concourse.bass_utils import run_bass_kernel_spmd

F32 = mybir.dt.float32
F16 = mybir.dt.float16
F8 = mybir.dt.float8e4

B, C, H, W = 16, 256, 64, 64
N = H * W                 # 4096
NCORES = 8
PB = B // NCORES          # batch items per core
P = 128                   # partitions
CT = C // P               # 2 channel tiles
NT = N // P               # 32 pixel tiles
CC = 258                  # per-pixel-tile row width: 256 ch + [1, 0]
CHUNKS0 = [2, 2, 4, 4, 4, 4, 4, 4, 4]      # item0 chunk sizes (nt tiles)
CHUNKS1 = [4, 4, 4, 4, 4, 4, 4, 4]          # item1 chunk sizes
FD = 512                  # free-dim per DoubleRow matmul (one PSUM bank)
OD = 2048                 # output store width (4 matmul blocks, 256KB fp8)
ASC = 16.0                # fp8 prescale for (attn wv); host divides out

# wpack column layout (fp16, packed on host into [128, WCOLS]):
_WQ0, _WQ1 = 0, 256              # wq^T rows 0:128 / 128:256   [128,256] each
_WK0, _WK1 = 512, 768            # wk^T rows 0:128 / 128:256
_WV0, _WV1 = 1024, 1282         # [wv | 16*bv | 0] rows 0:128/128:256 [128,258]
_BQ = 1540                       # rows 0:2: [bq; 0]            [2,256]
_BK = 1796                       # rows 0:2: [bk; 0]            [2,256]
_NC = 2052                       # rows 0:2: [float(N); 0]      [2,1]
_ID = 2056                       # fp16 identity [128,128]
WCOLS = 2184


def _emit_core_program(nc, tc, ctx, xt_in, x8_in, wpack, y_out):
    sb1 = ctx.enter_context(tc.tile_pool(name="sb1", bufs=1))
    xtp = ctx.enter_context(tc.tile_pool(name="xtp", bufs=1))
    x8p = ctx.enter_context(tc.tile_pool(name="x8p", bufs=PB))
    gsb = ctx.enter_context(tc.tile_pool(name="gsb", bufs=2 * PB))
    smp = ctx.enter_context(tc.tile_pool(name="smp", bufs=2 * PB))
    ysp = ctx.enter_context(tc.tile_pool(name="ysp", bufs=16))
    # PSUM pools: psg 2 + psb 2 + pso 4 = 8 banks
    psg = ctx.enter_context(tc.tile_pool(name="psg", bufs=2, space="PSUM"))
    psb = ctx.enter_context(tc.tile_pool(name="psb", bufs=2, space="PSUM"))
    pso = ctx.enter_context(tc.tile_pool(name="pso", bufs=4, space="PSUM"))

    # ---- all input DMAs on the sync queue, in need-order ----
    # xt[b] = list of (start_nt, n_nt, tile); graduated sizes so the first
    # gram matmuls start as early as possible
    xt = []
    for b, sizes in ((0, CHUNKS0), (1, CHUNKS1)):
        chunks, nt0 = [], 0
        for ci, n_nt in enumerate(sizes):
            t = xtp.tile([P, n_nt * CC], F16, tag=f"xt{b}_{ci}",
                         name=f"xt{b}_{ci}")
            chunks.append((nt0, n_nt, t))
            nt0 += n_nt
        xt.append(chunks)
    wt = sb1.tile([P, WCOLS], F16)
    x8s = [x8p.tile([P, CT, N], F8, tag="x8", name=f"x8_{b}")
           for b in range(PB)]

    def dma_xt(b):
        for (nt0, n_nt, t) in xt[b]:
            nc.sync.dma_start(
                out=t, in_=xt_in[b, :, nt0 * CC:(nt0 + n_nt) * CC])

    dma_xt(0)
    nc.sync.dma_start(out=wt, in_=wpack[:, :])
    dma_xt(1)
    for b in range(PB):
        for ct in range(CT):
            nc.sync.dma_start(out=x8s[b][:, ct, :],
                              in_=x8_in[b, ct * P:(ct + 1) * P, :])

    # ---- constants ----
    ident = wt[:, _ID:_ID + P]

    wq_k = [wt[:, _WQ0:_WQ0 + 256], wt[:, _WQ1:_WQ1 + 256],
            wt[0:2, _BQ:_BQ + 256]]
    wk_k = [wt[:, _WK0:_WK0 + 256], wt[:, _WK1:_WK1 + 256],
            wt[0:2, _BK:_BK + 256]]
    wv_t = [wt[:, _WV0:_WV0 + 258], wt[:, _WV1:_WV1 + 258]]

    st = [dict() for _ in range(PB)]

    def emit_gram(b):
        s = st[b]
        with nc.named_scope("gram"):
            gps0 = psg.tile([P, CC], F32, tag="g", name=f"gps{b}_0")
            gps1 = psg.tile([P, CC - P], F32, tag="g", name=f"gps{b}_1")
            for (nt0, n_nt, xc) in xt[b]:
                for off in range(n_nt):
                    nt = nt0 + off
                    nc.tensor.matmul(gps0, xc[:, off * CC:off * CC + P],
                                     xc[:, off * CC:(off + 1) * CC],
                                     start=(nt == 0), stop=(nt == NT - 1))
                    nc.tensor.matmul(
                        gps1, xc[:, off * CC + P:off * CC + 2 * P],
                        xc[:, off * CC + P:(off + 1) * CC],
                        start=(nt == 0), stop=(nt == NT - 1))
            s["gps"] = (gps0, gps1)

    def emit_gass(b):
        s = st[b]
        gps0, gps1 = s["gps"]
        with nc.named_scope("gass"):
            g0 = gsb.tile([P, CC], F16, tag="g", name=f"g{b}_0")
            nc.vector.tensor_copy(g0[:, P:2 * P], gps0[:, P:2 * P])
            nc.vector.tensor_copy(g0[:, 0:P], gps0[:, 0:P])
            nc.vector.tensor_copy(g0[:, 2 * P:CC], gps0[:, 2 * P:CC])
            g1 = gsb.tile([P, CC], F16, tag="g", name=f"g{b}_1")
            nc.vector.tensor_copy(g1[:, P:CC], gps1)
            tp10 = pso.tile([P, P], F16, tag="o", name=f"tp10_{b}")
            nc.tensor.transpose(tp10, g0[:, P:2 * P], ident)
            nc.vector.tensor_copy(g1[:, 0:P], tp10)
            g2 = gsb.tile([2, 257], F16, tag="g2", name=f"g2_{b}")
            for ct in range(CT):
                g = (g0, g1)[ct]
                sp = pso.tile([2, P], F16, tag="o", name=f"sp{b}_{ct}")
                nc.tensor.transpose(sp, g[:, 256:258], ident)
                nc.vector.tensor_copy(g2[0:2, ct * P:(ct + 1) * P], sp)
            nc.vector.tensor_copy(g2[0:2, 256:257], wt[0:2, _NC:_NC + 1])
            s["gk"] = (g0, g1, g2)

    def emit_TE(b):
        s = st[b]
        gk = s["gk"]
        with nc.named_scope("energy"):
            ttp = psb.tile([P, 2 * 256], F32, tag="big", name=f"ttp{b}")
            for mt in range(CT):
                for kt in range(3):
                    lhs = gk[kt][:, mt * P:(mt + 1) * P] if kt < 2 \
                        else gk[2][0:2, mt * P:(mt + 1) * P]
                    nc.tensor.matmul(ttp[:, mt * 256:(mt + 1) * 256],
                                     lhs, wq_k[kt],
                                     start=(kt == 0), stop=(kt == 2))
            tt2p = pso.tile([1, 256], F32, tag="o", name=f"tt2p{b}")
            for kt in range(3):
                lhs = gk[kt][:, 256:257] if kt < 2 else gk[2][0:2, 256:257]
                nc.tensor.matmul(tt2p, lhs, wq_k[kt],
                                 start=(kt == 0), stop=(kt == 2))
            tt = []
            for mt in range(CT):
                t = gsb.tile([P, 256], F16, tag="tt", name=f"tt{b}_{mt}")
                nc.vector.tensor_copy(t, ttp[:, mt * 256:(mt + 1) * 256])
                tt.append(t)
            t2 = gsb.tile([1, 256], F16, tag="tt2", name=f"tt2_{b}")
            nc.vector.tensor_copy(t2, tt2p)
            tt.append(t2)

            ep = psb.tile([P, 2 * 256], F32, tag="big", name=f"ep{b}")
            for it in range(CT):
                for kt in range(3):
                    lhs = tt[kt][:, it * P:(it + 1) * P] if kt < 2 \
                        else tt[2][0:1, it * P:(it + 1) * P]
                    nc.tensor.matmul(ep[:, it * 256:(it + 1) * 256],
                                     lhs, wk_k[kt][0:1, :] if kt == 2
                                     else wk_k[kt],
                                     start=(kt == 0), stop=(kt == 2))
            s["ep"] = ep

    def emit_softmax(b):
        s = st[b]
        ep = s["ep"]
        with nc.named_scope("softmax"):
            attn = []
            for it in range(CT):
                eslice = ep[:, it * 256:(it + 1) * 256]
                nmx = smp.tile([P, 1], F32, tag="nmx", name=f"nmx{b}_{it}")
                nc.vector.tensor_reduce(
                    nmx, eslice, axis=mybir.AxisListType.X,
                    op=mybir.AluOpType.max, negate=True)
                at = smp.tile([P, 256], F16, tag="attn", name=f"at{b}_{it}")
                rs = smp.tile([P, 1], F32, tag="rs", name=f"rs{b}_{it}")
                nc.scalar.activation(
                    out=at, in_=eslice,
                    func=mybir.ActivationFunctionType.Exp,
                    bias=nmx, scale=1.0, accum_out=rs)
                ri = smp.tile([P, 1], F32, tag="ri", name=f"ri{b}_{it}")
                nc.vector.reciprocal(ri, rs)
                nc.vector.tensor_scalar_mul(at, at, ri)
                attn.append(at)
            s["attn"] = attn

    def emit_attnT(b):
        s = st[b]
        attn = s["attn"]
        with nc.named_scope("softmax"):
            attnT = [smp.tile([P, 256], F16, tag="attnT", name=f"aT{b}_{jt}")
                     for jt in range(CT)]
            for it in range(CT):
                for jt in range(CT):
                    tp = pso.tile([P, P], F16, tag="o", name=f"tA{b}{jt}{it}")
                    nc.tensor.transpose(
                        tp, attn[it][:, jt * P:(jt + 1) * P], ident)
                    dst = attnT[jt][:, it * P:(it + 1) * P]
                    if (it + jt) % 2 == 0:
                        nc.scalar.copy(dst, tp)
                    else:
                        nc.vector.tensor_copy(dst, tp)
            s["attnT"] = attnT

    def emit_awv(b):
        s = st[b]
        attnT = s["attnT"]
        with nc.named_scope("attn_wv"):
            # at8[p, ct, o] = 16*(attn wv)[o, ct*128+p] in fp8
            at8 = smp.tile([P, CT, 256], F8, tag="at8", name=f"at8_{b}")
            ap_ = psb.tile([P, 2 * 256], F32, tag="big", name=f"ap{b}")
            for mt in range(CT):
                for jt in range(CT):
                    nc.tensor.matmul(
                        ap_[:, mt * 256:(mt + 1) * 256],
                        wv_t[jt][:, mt * P:(mt + 1) * P], attnT[jt],
                        start=(jt == 0), stop=(jt == 1))
                if mt == 0:
                    nc.scalar.mul(at8[:, mt, :],
                                  ap_[:, mt * 256:(mt + 1) * 256], ASC)
                else:
                    nc.vector.tensor_scalar_mul(
                        at8[:, mt, :], ap_[:, mt * 256:(mt + 1) * 256], ASC)
            # abv[it] = 16*(attn bv) column [128,1] (bv pre-scaled in wpack)
            abv = []
            for it in range(CT):
                avp = pso.tile([P, 1], F32, tag="o", name=f"avp{b}_{it}")
                for jt in range(CT):
                    nc.tensor.matmul(avp,
                                     attnT[jt][:, it * P:(it + 1) * P],
                                     wv_t[jt][:, 256:257],
                                     start=(jt == 0), stop=(jt == 1))
                ac = smp.tile([P, 1], F32, tag="abv", name=f"abv{b}_{it}")
                nc.scalar.copy(ac, avp)
                abv.append(ac)
            s["at8"], s["abv"] = at8, abv

    def emit_out(b):
        s = st[b]
        at8, abv, x8 = s["at8"], s["abv"], x8s[b]
        with nc.named_scope("out_mm"):
            for it in range(CT):
                lhsT = at8[:, :, it * P:(it + 1) * P]
                for og in range(N // OD):
                    ysb = ysp.tile([P, OD], F8, tag="ysb",
                                   name=f"ysb{b}_{it}_{og}")
                    for h in range(OD // FD):
                        nch = og * (OD // FD) + h
                        op = pso.tile([P, FD], F32, tag="o",
                                      name=f"op{b}_{it}_{nch}")
                        nc.tensor.matmul(
                            op, lhsT,
                            x8[:, :, nch * FD:(nch + 1) * FD],
                            start=True, stop=True,
                            perf_mode=mybir.MatmulPerfMode.DoubleRow)
                        dst = ysb[:, h * FD:(h + 1) * FD]
                        if h % 2 == 0:
                            nc.scalar.add(dst, op, add=abv[it])
                        else:
                            nc.vector.tensor_scalar_add(dst, op, abv[it])
                    nc.sync.dma_start(
                        out=y_out[b, it * P:(it + 1) * P,
                                  og * OD:(og + 1) * OD],
                        in_=ysb)

    # ---- pipelined emission across the two items ----
    emit_gram(0)
    emit_gass(0)
    emit_TE(0)
    emit_softmax(0)     # DVE/ACT only; hides under item1's gram
    emit_gram(1)
    emit_attnT(0)       # PE transposes run right after gram1
    emit_awv(0)
    emit_gass(1)
    emit_TE(1)
    emit_softmax(1)     # hides under item0's output matmuls
    emit_out(0)
    emit_attnT(1)
    emit_awv(1)
    emit_out(1)


_CACHE = {}
LAST_RESULTS = None


def _build():
    if "nc" in _CACHE:
        return _CACHE["nc"]
    nc = bacc.Bacc()
    xt_in = nc.declare_dram_parameter("xt", [PB, P, NT * CC], F16,
                                      isOutput=False)
    x8_in = nc.declare_dram_parameter("x8", [PB, C, N], F8, isOutput=False)
    wpack = nc.declare_dram_parameter("wpack", [P, WCOLS], F16,
                                      isOutput=False)
    y_out = nc.declare_dram_parameter("y", [PB, C, N], F8, isOutput=True)
    with ExitStack() as ctx:
        tc = ctx.enter_context(tile.TileContext(nc))
        _emit_core_program(nc, tc, ctx, xt_in, x8_in, wpack, y_out)
    nc.compile()
    _CACHE["nc"] = nc
    return nc


def _pack_weights(wq, bq, wk, bk, wv, bv):
    wp = np.zeros((P, WCOLS), np.float16)
    wqT = np.ascontiguousarray(wq.T).astype(np.float16)
    wkT = np.ascontiguousarray(wk.T).astype(np.float16)
    wp[:, _WQ0:_WQ0 + 256] = wqT[0:P]
    wp[:, _WQ1:_WQ1 + 256] = wqT[P:C]
    wp[:, _WK0:_WK0 + 256] = wkT[0:P]
    wp[:, _WK1:_WK1 + 256] = wkT[P:C]
    wvp = np.concatenate([wv, ASC * bv[:, None]],
                         axis=1).astype(np.float16)  # [256, 257]
    wp[:, _WV0:_WV0 + 257] = wvp[0:P]
    wp[:, _WV1:_WV1 + 257] = wvp[P:C]
    wp[0, _BQ:_BQ + 256] = bq.astype(np.float16)
    wp[0, _BK:_BK + 256] = bk.astype(np.float16)
    wp[0, _NC] = np.float16(float(N))
    wp[:, _ID:_ID + P] = np.eye(P, dtype=np.float16)
    return wp


def kernel(x, wq, bq, wk, bk, wv, bv, gamma):
    global LAST_RESULTS
    x = np.ascontiguousarray(np.asarray(x, np.float32).reshape(B, C, N))
    x16 = x.astype(np.float16)
    # xt[b, p, nt, c] = x[b, c, nt*128+p]; col 256 = 1.0, col 257 = 0.0
    xt = np.zeros((B, P, NT, CC), np.float16)
    xt[:, :, :, :256] = x16.reshape(B, C, NT, P).transpose(0, 3, 2, 1)
    xt[:, :, :, 256] = np.float16(1.0)
    xt = np.ascontiguousarray(xt.reshape(B, P, NT * CC))
    x8 = x.astype(ml_dtypes.float8_e4m3)
    wp = _pack_weights(np.asarray(wq, np.float32), np.asarray(bq, np.float32),
                       np.asarray(wk, np.float32), np.asarray(bk, np.float32),
                       np.asarray(wv, np.float32), np.asarray(bv, np.float32))
    nc = _build()
    in_maps = []
    for k in range(NCORES):
        in_maps.append({
            "xt": np.ascontiguousarray(xt[k * PB:(k + 1) * PB]),
            "x8": np.ascontiguousarray(x8[k * PB:(k + 1) * PB]),
            "wpack": wp,
        })
    trace = bool(int(os.environ.get("KERNEL_TRACE", "0")))
    res = run_bass_kernel_spmd(nc, in_maps, core_ids=list(range(NCORES)),
                               trace=trace)
    LAST_RESULTS = res
    yd = np.concatenate([np.asarray(res.results[k]["y"])[None]
                         for k in range(NCORES)], axis=0).reshape(B, C, N)
    g = float(np.asarray(gamma, np.float32).reshape(-1)[0])
    y = (g / ASC) * yd.astype(np.float32) + x
    return y.reshape(B, C, H, W)
